# revision 49
# baseline (speedup 1.0000x reference)
"""Trainium2 Bass kernel for nn_Attention (dense transformer cross-attention).

Strategy: data-parallel over batch (B=8) -> one batch element per NeuronCore.
Per core, everything is computed with zero on-chip transposes by choosing
layouts up front (host pre-transposes activations/weights, which is free):

  K^T_h [dh=128, Mk]  = Wk-chunk^T . memory^T   (per head h, bias via ACT)
  Q^T_h [dh=128, Q]   = (scaled Wq)-chunk^T . query^T
  V     [Mk, D]       = memory . Wv^T           (bv folded into bf on host:
                        softmax rows sum to 1, so + bv passes through PV)
  S^T   [Mk, Q]       = K^T_h^T-free-slices . Q^T_h    (heads pipelined)
  expS  = ACT Exp (per-partition mask bias in the general path), bf16
  sum_q = incremental bf16 pair tree over m-chunks (DVE) + ones-row matmul
          (partition-direction sum on PE = free broadcast)
  1/sum = exp(-ln(x)) on ACT, ~6x faster than DVE RECIPROCAL and off the
          in-order DVE queue that carries the ctx drains; emitted DEFERRED
          one head later so its PE/PSUM use never blocks the score stream
  ctx^T_h [dh, Q]     = V-chunks . expS  (PSUM accum over m-chunks, 3 banks
                        so drains may lag a full head window),
                        normalization fused into the PSUM->SBUF drain
  out   [Q, D]        = ctx^T (as lhsT, heads = contraction chunks) . Wf^T
                        + bf' (fused into final drain)

Fast path: the reference mask is fixed (last quarter of memory positions
masked for every batch) -> those m-positions have exactly-zero softmax
weight, so the masked chunks are skipped outright in K/V projections,
scores, exp, and PV; their slice of the returned attention map is zeros.
Host prep verifies the mask and falls back to a general program (mask as
per-partition exp bias, full M) for any other mask.

Phase overlap: Q's inputs live in the persist pool, so phase 2's SBUF
allocations only wait on the K/V projections and the Q projection overlaps
the first attention heads. Inputs stream chunkwise in consumption order
over the three DMA queues (per-queue bandwidth ~130GB/s is the wall, not
descriptor size; V's inputs go first and V is computed first).

Softmax max-subtraction is skipped: scores are O(1) by construction
(0.02-scale weights), exp is computed in f32 on ACT, so this is exact.

Compute dtype bf16 (f32 PSUM accumulation); inputs/outputs bf16 so DMA
moves half the bytes (outputs are upcast on host).
"""

import math

import numpy as np
import ml_dtypes

B = 8
Q = 1024
M = 1024
D = 1024
H = 8
DH = 128
KC = 8   # 128-row contraction chunks per 1024
NT = 2   # 512-wide free tiles per 1024
FT = 512
MK_FAST = 768   # unmasked memory positions in the canonical mask
AKC_FAST = 6    # active m-chunks in the fast path

_BF16 = ml_dtypes.bfloat16
_CACHE = {}


def _build_program(reps=1, parts='paf', norm=True, fast=True):
    import concourse.bass as bass
    import concourse.mybir as mybir
    from concourse.tile import TileContext

    import bass_rust

    f32 = mybir.dt.float32
    bf16 = mybir.dt.bfloat16
    Identity = mybir.ActivationFunctionType.Identity
    Exp = mybir.ActivationFunctionType.Exp
    Ln = mybir.ActivationFunctionType.Ln

    MKC = AKC_FAST if fast else KC     # active m-chunks
    MKW = MKC * DH                     # active memory positions
    # free-dim tiles covering MKW (512-wide, last may be short)
    m_tiles = []
    off = 0
    while off < MKW:
        w = min(FT, MKW - off)
        m_tiles.append(slice(off, off + w))
        off += w

    def split_sync_waits(nc):
        """The walrus in this container accepts only ONE sync-wait per
        instruction; Tile freely attaches several. Move excess waits onto
        same-engine NOPs spliced immediately before the instruction."""
        for fn in nc.m.functions:
            for bb in fn.blocks:
                out = []
                for inst in bb.instructions:
                    si = inst.sync_info
                    if si is not None and si.on_wait is not None and len(si.on_wait) > 1:
                        waits = list(si.on_wait)
                        si.on_wait = waits[-1:]
                        for j, w in enumerate(waits[:-1]):
                            nop = bass_rust.InstNoOp(
                                name=f"{inst.name}_sw{j}", ins=[], outs=[])
                            nop.engine = inst.engine
                            nop.sync_info = mybir.SyncInfo(on_wait=[w], on_update=[])
                            out.append(nop)
                    out.append(inst)
                bb.instructions = out

    nc = bass.Bass()

    # all [contraction=D, N] operands arrive pre-shuffled on host to
    # partition-major [128, KC, N]: each SBUF partition's data is one
    # contiguous 16KB DRAM run -> 8x larger DMA descriptors
    memT = nc.declare_dram_parameter("memT", [128, KC, MKW], bf16, isOutput=False)
    qT = nc.declare_dram_parameter("qT", [128, KC, Q], bf16, isOutput=False)
    wkT = nc.declare_dram_parameter("wkT", [128, KC, D], bf16, isOutput=False)
    wvT = nc.declare_dram_parameter("wvT", [128, KC, D], bf16, isOutput=False)
    wqT = nc.declare_dram_parameter("wqT", [128, KC, D], bf16, isOutput=False)
    wfT = nc.declare_dram_parameter("wfT", [128, KC, D], bf16, isOutput=False)
    bk_pp = nc.declare_dram_parameter("bk_pp", [128, H], f32, isOutput=False)
    bq_pp = nc.declare_dram_parameter("bq_pp", [128, H], f32, isOutput=False)
    if not fast:
        mb_pp = nc.declare_dram_parameter("mb_pp", [128, KC], f32, isOutput=False)
    bf_bc = nc.declare_dram_parameter("bf_bc", [128, D], f32, isOutput=False)

    wm = nc.declare_dram_parameter("wm", [Q, D], bf16, isOutput=True)
    p0t = nc.declare_dram_parameter("p0t", [MKW, Q], bf16, isOutput=True)

    with TileContext(nc) as tc:
      for rep in range(reps):
        with tc.tile_pool(name=f"const{rep}", bufs=1) as const, \
             tc.tile_pool(name=f"persist{rep}", bufs=1) as persist:
            wf_sb = const.tile([128, KC, D], bf16)
            bkt = const.tile([128, H], f32)
            bqt = const.tile([128, H], f32)
            if not fast:
                mbt = const.tile([128, KC], f32)
            bft = const.tile([128, D], f32)
            ones128 = const.tile([128, 128], bf16)

            nc.scalar.dma_start(out=bkt[:], in_=bk_pp[:, :])
            nc.scalar.dma_start(out=bqt[:], in_=bq_pp[:, :])
            if not fast:
                nc.scalar.dma_start(out=mbt[:], in_=mb_pp[:, :])
            nc.scalar.dma_start(out=bft[:], in_=bf_bc[:, :])
            nc.vector.memset(ones128[:], 1.0)

            k_sb = persist.tile([128, H, MKW], bf16)
            q_sb = persist.tile([128, H, Q], bf16)
            v_sb = persist.tile([128, MKC, D], bf16)
            ctx_sb = persist.tile([128, H, Q], bf16)
            # Q inputs live in persist so the per-head Q projection can be
            # fused into the phase-2 loop (its PE work hides under phase 2's
            # ACT-bound head windows)
            qt_sb = persist.tile([128, KC, Q], bf16)
            wq_sb = persist.tile([128, KC, D], bf16)

            # ---------------- Phase 1: K/V projections ----------------
            with tc.tile_pool(name=f"proj{rep}", bufs=1) as proj, \
                 tc.tile_pool(name=f"ppsum{rep}", bufs=4, space="PSUM") as ppsum:
                mem_sb = proj.tile([128, KC, MKW], bf16)
                wk_sb = proj.tile([128, KC, D], bf16)
                wv_sb = proj.tile([128, KC, D], bf16)
                # per-queue DMA bandwidth (~130GB/s) is the wall at kernel
                # start. V's inputs (mem on scalar, wv on gpsimd) land
                # fastest, so V runs FIRST; wk streams concurrently on sync
                # and is resident by the time K starts; Q inputs queue FIFO
                # behind and are consumed inside phase 2.
                for c in range(KC):
                    nc.scalar.dma_start(out=mem_sb[:, c, :], in_=memT[:, c, :])
                    nc.sync.dma_start(out=wv_sb[:, c, :], in_=wvT[:, c, :])
                nc.gpsimd.dma_start(out=wk_sb[:], in_=wkT[:, :, :])
                nc.scalar.dma_start(out=wq_sb[:], in_=wqT[:, :, :])
                nc.sync.dma_start(out=qt_sb[:], in_=qT[:, :, :])
                nc.gpsimd.dma_start(out=wf_sb[:], in_=wfT[:, :, :])

                for mc in range(MKC):
                    ms = slice(mc * DH, (mc + 1) * DH)
                    ps = ppsum.tile([128, Q], f32, tag="pp")
                    for c in range(KC):
                        for t in range(NT):
                            ts_ = slice(t * FT, (t + 1) * FT)
                            nc.tensor.matmul(
                                ps[:, ts_], mem_sb[:, c, ms], wv_sb[:, c, ts_],
                                start=(c == 0), stop=(c == KC - 1))
                    # bv folded into bf' on host -> pure copy drain (ACT is
                    # idle in phase 1; GpSimd cannot read PSUM)
                    nc.scalar.activation(v_sb[:, mc, :], ps[:], Identity)
                for h in range(H):
                    hs = slice(h * DH, (h + 1) * DH)
                    ps = ppsum.tile([128, Q], f32, tag="pp")
                    for c in range(KC):
                        for ts_ in m_tiles:
                            nc.tensor.matmul(
                                ps[:, ts_], wk_sb[:, c, hs], mem_sb[:, c, ts_],
                                start=(c == 0), stop=(c == KC - 1))
                    nc.scalar.activation(
                        k_sb[:, h, :], ps[:, :MKW], Identity, bias=bkt[:, h:h + 1])
                for h in range(H):
                    hs = slice(h * DH, (h + 1) * DH)
                    ps2 = ppsum.tile([128, Q], f32, tag="pp")
                    for c in range(KC):
                        for t in range(NT):
                            ts_ = slice(t * FT, (t + 1) * FT)
                            nc.tensor.matmul(
                                ps2[:, ts_], wq_sb[:, c, hs], qt_sb[:, c, ts_],
                                start=(c == 0), stop=(c == KC - 1))
                    nc.scalar.activation(
                        q_sb[:, h, :], ps2[:], Identity, bias=bqt[:, h:h + 1])

            # ---------------- Phase 2: attention (per head) ----------------
            if 'a' not in parts:
                continue
            with tc.tile_pool(name=f"attn{rep}", bufs=2) as attn, \
                 tc.tile_pool(name=f"attn3{rep}", bufs=3) as attn3, \
                 tc.tile_pool(name=f"dramp{rep}", bufs=2, space="DRAM") as dramp, \
                 tc.tile_pool(name=f"spsum{rep}", bufs=2, space="PSUM") as spsum, \
                 tc.tile_pool(name=f"cpsum{rep}", bufs=3, space="PSUM") as cpsum, \
                 tc.tile_pool(name=f"upsum{rep}", bufs=1, space="PSUM") as upsum:
                def emit_pv(ph, pexp, prb, cp_tiles, idx):
                    # one PV matmul of the software-pipelined previous head;
                    # idx walks cc-major: cc = idx // NT, t = idx % NT so the
                    # stationary V chunk is reused by consecutive matmuls
                    cc, t_ = divmod(idx, NT)
                    ts_ = slice(t_ * FT, (t_ + 1) * FT)
                    phs = slice(ph * DH, (ph + 1) * DH)
                    if idx == 0:
                        for t in range(NT):
                            cp_tiles[t] = cpsum.tile([128, FT], f32, tag="cp",
                                                     name=f"cp_h{ph}_t{t}")
                    nc.tensor.matmul(
                        cp_tiles[t_][:], v_sb[:, cc, phs], pexp[:, cc, ts_],
                        start=(cc == 0), stop=(cc == MKC - 1))
                    if cc == MKC - 1:
                        if norm:
                            nc.vector.tensor_mul(
                                ctx_sb[:, ph, ts_], cp_tiles[t_][:], prb[:, ts_])
                        else:
                            nc.vector.tensor_copy(ctx_sb[:, ph, ts_], cp_tiles[t_][:])

                def emit_p0(pexp, prb):
                    for c in range(MKC):
                        p0_sb = attn3.tile([128, Q], bf16, tag="p0")
                        nc.gpsimd.tensor_mul(p0_sb[:], pexp[:, c, :], prb[:])
                        nc.sync.dma_start(
                            out=p0t[c * DH:(c + 1) * DH, :], in_=p0_sb[:])

                def emit_sums(acc, rb_sb, h):
                    # deferred to the next head's window: the single upsum
                    # bank and the PE ones-matmuls then never block this
                    # head's own score stream in the in-order queues.
                    # 1/sum = exp(-ln(x)) on ACT (2 LUT passes, ~6x faster
                    # than DVE RECIPROCAL, which would also sit in front of
                    # the ctx drains in the in-order DVE queue).
                    for t in range(NT):
                        ts_ = slice(t * FT, (t + 1) * FT)
                        sum_bc = upsum.tile([128, FT], f32, tag="sum",
                                            name=f"sum_h{h}_t{t}")
                        nc.tensor.matmul(
                            sum_bc[:], ones128[:], acc[:, ts_],
                            start=True, stop=True)
                        ln_t = attn.tile([128, FT], f32, tag="ln")
                        nc.scalar.activation(ln_t[:], sum_bc[:], Ln)
                        nc.scalar.activation(
                            rb_sb[:, ts_], ln_t[:], Exp, scale=-1.0)

                prev = None       # (h, exp_sb, rb_sb)
                pend_sums = None  # (acc, rb_sb, h) for the previous head
                NP = MKC // 2  # exp chunk pairs
                for h in range(H):
                    hs = slice(h * DH, (h + 1) * DH)
                    exp_sb = attn.tile([128, MKC, Q], bf16, tag="expS",
                                       bufs=3 if fast else 2,
                                       name=f"exp_h{h}")
                    if norm:
                        red1 = attn.tile([128, NP, Q], bf16, tag="red1")
                    cp_tiles = [None, None]
                    for c in range(MKC):
                        cs = slice(c * DH, (c + 1) * DH)
                        st = spsum.tile([128, Q], f32, tag="st")
                        for t in range(NT):
                            ts_ = slice(t * FT, (t + 1) * FT)
                            nc.tensor.matmul(
                                st[:, ts_], k_sb[:, h, cs], q_sb[:, h, ts_],
                                start=True, stop=True)
                        if fast:
                            nc.scalar.activation(exp_sb[:, c, :], st[:], Exp)
                        else:
                            nc.scalar.activation(
                                exp_sb[:, c, :], st[:], Exp, bias=mbt[:, c:c + 1])
                        if norm and c % 2 == 1:
                            # incremental tree level 1: sum this exp pair as
                            # soon as both chunks exist (shortens the rb
                            # critical path after the last exp)
                            nc.vector.tensor_add(
                                red1[:, c // 2, :], exp_sb[:, c - 1, :],
                                exp_sb[:, c, :])
                        if c == 0 and pend_sums is not None:
                            emit_sums(*pend_sums)
                            pend_sums = None
                        if prev is not None:
                            # interleave prev head's PV so PE stays busy (and
                            # HAM-warm) through the ACT-bound exp stretch
                            emit_pv(prev[0], prev[1], prev[2], cp_tiles, 2 * c)
                            emit_pv(prev[0], prev[1], prev[2], cp_tiles, 2 * c + 1)
                    if prev is not None and prev[0] == 0 and norm:
                        emit_p0(prev[1], prev[2])
                    if norm:
                        # finish the pair tree, then a ones[128,128]-stationary
                        # matmul: every output partition gets the
                        # cross-partition sum, i.e. broadcast comes free.
                        if NP == 3:
                            red2 = attn.tile([128, Q], bf16, tag="red2")
                            nc.vector.tensor_add(
                                red2[:], red1[:, 0, :], red1[:, 1, :])
                            acc = attn.tile([128, Q], bf16, tag="acc")
                            nc.vector.tensor_add(acc[:], red2[:], red1[:, 2, :])
                        else:
                            red2 = attn.tile([128, 2, Q], bf16, tag="red2")
                            nc.vector.tensor_add(
                                red2[:], red1[:, 0:4:2, :], red1[:, 1:4:2, :])
                            acc = attn.tile([128, Q], bf16, tag="acc")
                            nc.vector.tensor_add(acc[:], red2[:, 0, :], red2[:, 1, :])
                        rb_sb = attn.tile([128, Q], f32, tag="rb", name=f"rb_h{h}")
                        pend_sums = (acc, rb_sb, h)
                    else:
                        rb_sb = None
                    prev = (h, exp_sb, rb_sb)

                # drain the last head: its sums first, then its PV
                if pend_sums is not None:
                    emit_sums(*pend_sums)
                cp_tiles = [None, None]
                for idx in range(NT * MKC):
                    emit_pv(prev[0], prev[1], prev[2], cp_tiles, idx)

            # ---------------- Phase 3: final projection ----------------
            if 'f' not in parts:
                continue
            with tc.tile_pool(name=f"fin{rep}", bufs=3) as fin, \
                 tc.tile_pool(name=f"fpsum{rep}", bufs=3, space="PSUM") as fpsum:
                for qc in range(KC):
                    qs = slice(qc * DH, (qc + 1) * DH)
                    for t in range(NT):
                        ts_ = slice(t * FT, (t + 1) * FT)
                        fp = fpsum.tile([128, FT], f32, tag="fp")
                        for h in range(H):
                            nc.tensor.matmul(
                                fp[:], ctx_sb[:, h, qs], wf_sb[:, h, ts_],
                                start=(h == 0), stop=(h == H - 1))
                        of = fin.tile([128, FT], bf16, tag="of")
                        nc.vector.tensor_add(of[:], fp[:], bft[:, ts_])
                        eng = nc.scalar if (qc + t) % 2 == 0 else nc.sync
                        eng.dma_start(out=wm[qs, ts_], in_=of[:])

    split_sync_waits(nc)
    return nc


def _get_program(reps=1, parts='paf', norm=True, fast=True):
    key = f"nc{reps}_{parts}_{norm}_{fast}"
    if key not in _CACHE:
        _CACHE[key] = _build_program(reps, parts, norm, fast)
    return _CACHE[key]


def _is_canonical_mask(mask):
    mask = np.asarray(mask)
    want = np.broadcast_to(np.arange(M)[None, :] >= MK_FAST, (B, M))
    return mask.shape == (B, M) and bool(np.array_equal(mask, want))


def _host_prep(query, memory, mask, Wk, bk, Wv, bv, Wq, bq, Wf, bf, fast):
    scale = 1.0 / math.sqrt(DH)
    f32 = np.float32
    mkw = MK_FAST if fast else M

    def pcn(t):
        # [D, N] -> partition-major [128, KC, N]: per-SBUF-partition data is
        # one contiguous DRAM run -> large DMA descriptors
        n = t.shape[1]
        return np.ascontiguousarray(
            t.reshape(KC, 128, n).transpose(1, 0, 2))

    def t_bf16(a):
        return pcn(np.asarray(a, dtype=f32).T.astype(_BF16))

    # bv folds through PV (softmax rows sum to 1): bf' = bf + Wf @ bv
    bf_eff = np.asarray(bf, dtype=f32) + (
        np.asarray(Wf, dtype=f32) @ np.asarray(bv, dtype=f32))

    shared = {
        "wkT": t_bf16(Wk),
        "wvT": t_bf16(Wv),
        "wqT": pcn((np.asarray(Wq, dtype=f32).T * f32(scale)).astype(_BF16)),
        "wfT": t_bf16(Wf),
        "bk_pp": np.ascontiguousarray(
            np.asarray(bk, dtype=f32).reshape(H, DH).T),
        "bq_pp": np.ascontiguousarray(
            (np.asarray(bq, dtype=f32) * f32(scale)).reshape(H, DH).T),
        "bf_bc": np.ascontiguousarray(
            np.broadcast_to(bf_eff, (128, D))),
    }
    mask = np.asarray(mask)
    in_maps = []
    for b in range(B):
        im = {
            **shared,
            "memT": t_bf16(np.asarray(memory[b])[:mkw]),
            "qT": t_bf16(query[b]),
        }
        if not fast:
            mb = np.where(mask[b], f32(-1e30), f32(0.0)).astype(f32)
            im["mb_pp"] = np.ascontiguousarray(mb.reshape(KC, DH).T)
        in_maps.append(im)
    return in_maps


def kernel(query, memory, mask, Wk, bk, Wv, bv, Wq, bq, Wf, bf):
    from concourse.bass_utils import run_bass_kernel_spmd

    fast = _is_canonical_mask(mask)
    nc = _get_program(fast=fast)
    in_maps = _host_prep(query, memory, mask, Wk, bk, Wv, bv, Wq, bq, Wf, bf,
                         fast)
    res = run_bass_kernel_spmd(nc, in_maps, core_ids=list(range(B)))
    mkw = MK_FAST if fast else M
    wm = np.stack([res.results[b]["wm"] for b in range(B)]).astype(np.float32)
    w0 = np.zeros((B, Q, M), dtype=np.float32)
    for b in range(B):
        w0[b, :, :mkw] = res.results[b]["p0t"].T.astype(np.float32)
    return wm, w0


# revision 51
# speedup vs baseline: 1.0094x; 1.0094x over previous
"""Trainium2 Bass kernel for nn_Attention (dense transformer cross-attention).

Strategy: data-parallel over batch (B=8) -> one batch element per NeuronCore.
Per core, everything is computed with zero on-chip transposes by choosing
layouts up front (host pre-transposes activations/weights, which is free):

  K^T_h [dh=128, Mk]  = Wk-chunk^T . memory^T   (per head h, bias via ACT)
  Q^T_h [dh=128, Q]   = (scaled Wq)-chunk^T . query^T
  V     [Mk, D]       = memory . Wv^T           (bv folded into bf on host:
                        softmax rows sum to 1, so + bv passes through PV)
  S^T   [Mk, Q]       = K^T_h^T-free-slices . Q^T_h    (heads pipelined)
  expS  = ACT Exp (per-partition mask bias in the general path), bf16
  sum_q = incremental bf16 pair tree over m-chunks (DVE) + ones-row matmul
          (partition-direction sum on PE = free broadcast)
  1/sum = exp(-ln(x)) on ACT, ~6x faster than DVE RECIPROCAL and off the
          in-order DVE queue that carries the ctx drains; emitted DEFERRED
          one head later so its PE/PSUM use never blocks the score stream
  ctx^T_h [dh, Q]     = V-chunks . expS  (PSUM accum over m-chunks, 3 banks
                        so drains may lag a full head window),
                        normalization fused into the PSUM->SBUF drain
  out   [Q, D]        = ctx^T (as lhsT, heads = contraction chunks) . Wf^T
                        + bf' (fused into final drain)

Fast path: the reference mask is fixed (last quarter of memory positions
masked for every batch) -> those m-positions have exactly-zero softmax
weight, so the masked chunks are skipped outright in K/V projections,
scores, exp, and PV; their slice of the returned attention map is zeros.
Host prep verifies the mask and falls back to a general program (mask as
per-partition exp bias, full M) for any other mask.

Phase overlap: Q's inputs live in the persist pool, so phase 2's SBUF
allocations only wait on the K/V projections and the Q projection overlaps
the first attention heads. Inputs stream chunkwise in consumption order
over the three DMA queues (per-queue bandwidth ~130GB/s is the wall, not
descriptor size; V's inputs go first and V is computed first).

Softmax max-subtraction is skipped: scores are O(1) by construction
(0.02-scale weights), exp is computed in f32 on ACT, so this is exact.

Compute dtype bf16 (f32 PSUM accumulation); inputs/outputs bf16 so DMA
moves half the bytes (outputs are upcast on host).
"""

import math

import numpy as np
import ml_dtypes

B = 8
Q = 1024
M = 1024
D = 1024
H = 8
DH = 128
KC = 8   # 128-row contraction chunks per 1024
NT = 2   # 512-wide free tiles per 1024
FT = 512
MK_FAST = 768   # unmasked memory positions in the canonical mask
AKC_FAST = 6    # active m-chunks in the fast path

_BF16 = ml_dtypes.bfloat16
_CACHE = {}


def _build_program(reps=1, parts='paf', norm=True, fast=True):
    import concourse.bass as bass
    import concourse.mybir as mybir
    from concourse.tile import TileContext

    import bass_rust

    f32 = mybir.dt.float32
    bf16 = mybir.dt.bfloat16
    Identity = mybir.ActivationFunctionType.Identity
    Exp = mybir.ActivationFunctionType.Exp
    Ln = mybir.ActivationFunctionType.Ln

    MKC = AKC_FAST if fast else KC     # active m-chunks
    MKW = MKC * DH                     # active memory positions
    # free-dim tiles covering MKW (512-wide, last may be short)
    m_tiles = []
    off = 0
    while off < MKW:
        w = min(FT, MKW - off)
        m_tiles.append(slice(off, off + w))
        off += w

    def split_sync_waits(nc):
        """The walrus in this container accepts only ONE sync-wait per
        instruction; Tile freely attaches several. Move excess waits onto
        same-engine NOPs spliced immediately before the instruction."""
        for fn in nc.m.functions:
            for bb in fn.blocks:
                out = []
                for inst in bb.instructions:
                    si = inst.sync_info
                    if si is not None and si.on_wait is not None and len(si.on_wait) > 1:
                        waits = list(si.on_wait)
                        si.on_wait = waits[-1:]
                        for j, w in enumerate(waits[:-1]):
                            nop = bass_rust.InstNoOp(
                                name=f"{inst.name}_sw{j}", ins=[], outs=[])
                            nop.engine = inst.engine
                            nop.sync_info = mybir.SyncInfo(on_wait=[w], on_update=[])
                            out.append(nop)
                    out.append(inst)
                bb.instructions = out

    nc = bass.Bass()

    # all [contraction=D, N] operands arrive pre-shuffled on host to
    # partition-major [128, KC, N]: each SBUF partition's data is one
    # contiguous 16KB DRAM run -> 8x larger DMA descriptors
    memT = nc.declare_dram_parameter("memT", [128, KC, MKW], bf16, isOutput=False)
    qT = nc.declare_dram_parameter("qT", [128, KC, Q], bf16, isOutput=False)
    wkT = nc.declare_dram_parameter("wkT", [128, KC, D], bf16, isOutput=False)
    wvT = nc.declare_dram_parameter("wvT", [128, KC, D], bf16, isOutput=False)
    wqT = nc.declare_dram_parameter("wqT", [128, KC, D], bf16, isOutput=False)
    wfT = nc.declare_dram_parameter("wfT", [128, KC, D], bf16, isOutput=False)
    bk_pp = nc.declare_dram_parameter("bk_pp", [128, H], f32, isOutput=False)
    bq_pp = nc.declare_dram_parameter("bq_pp", [128, H], f32, isOutput=False)
    if not fast:
        mb_pp = nc.declare_dram_parameter("mb_pp", [128, KC], f32, isOutput=False)
    bf_bc = nc.declare_dram_parameter("bf_bc", [128, D], f32, isOutput=False)

    wm = nc.declare_dram_parameter("wm", [Q, D], bf16, isOutput=True)
    p0t = nc.declare_dram_parameter("p0t", [MKW, Q], bf16, isOutput=True)

    with TileContext(nc) as tc:
      for rep in range(reps):
        with tc.tile_pool(name=f"const{rep}", bufs=1) as const, \
             tc.tile_pool(name=f"persist{rep}", bufs=1) as persist:
            wf_sb = const.tile([128, KC, D], bf16)
            bkt = const.tile([128, H], f32)
            bqt = const.tile([128, H], f32)
            if not fast:
                mbt = const.tile([128, KC], f32)
            bft = const.tile([128, D], f32)
            ones128 = const.tile([128, 128], bf16)

            nc.scalar.dma_start(out=bkt[:], in_=bk_pp[:, :])
            nc.scalar.dma_start(out=bqt[:], in_=bq_pp[:, :])
            if not fast:
                nc.scalar.dma_start(out=mbt[:], in_=mb_pp[:, :])
            nc.scalar.dma_start(out=bft[:], in_=bf_bc[:, :])
            nc.vector.memset(ones128[:], 1.0)

            k_sb = persist.tile([128, H, MKW], bf16)
            q_sb = persist.tile([128, H, Q], bf16)
            v_sb = persist.tile([128, MKC, D], bf16)
            ctx_sb = persist.tile([128, H, Q], bf16)
            # Q inputs live in persist so the per-head Q projection can be
            # fused into the phase-2 loop (its PE work hides under phase 2's
            # ACT-bound head windows)
            qt_sb = persist.tile([128, KC, Q], bf16)
            wq_sb = persist.tile([128, KC, D], bf16)

            # ---------------- Phase 1: K/V projections ----------------
            with tc.tile_pool(name=f"proj{rep}", bufs=1) as proj, \
                 tc.tile_pool(name=f"ppsum{rep}", bufs=2, space="PSUM") as ppsum:
                mem_sb = proj.tile([128, KC, MKW], bf16)
                wk_sb = proj.tile([128, KC, D], bf16)
                wv_sb = proj.tile([128, KC, D], bf16)
                # at kernel start all 8 cores slam shared HBM at once, so
                # DMA priority is everything: ONLY V's inputs go first (mem
                # on scalar, wv on sync in column halves so the t=0 sweep
                # starts after 1MB); wk/qT and wq/wf queue FIFO behind them
                # and arrive during V/K compute. The gpsimd queue is unused
                # (slow swdge ring that would steal early bandwidth).
                for c in range(KC):
                    nc.scalar.dma_start(out=mem_sb[:, c, :], in_=memT[:, c, :])
                    nc.sync.dma_start(out=wv_sb[:, c, 0:FT], in_=wvT[:, c, 0:FT])
                for c in range(KC):
                    nc.sync.dma_start(out=wv_sb[:, c, FT:D], in_=wvT[:, c, FT:D])
                nc.sync.dma_start(out=wk_sb[:], in_=wkT[:, :, :])
                nc.scalar.dma_start(out=wq_sb[:], in_=wqT[:, :, :])
                nc.sync.dma_start(out=qt_sb[:], in_=qT[:, :, :])
                nc.scalar.dma_start(out=wf_sb[:], in_=wfT[:, :, :])

                # t-outer V sweep: each output-column half only needs the
                # matching wv half resident
                for t in range(NT):
                    ts_ = slice(t * FT, (t + 1) * FT)
                    for mc in range(MKC):
                        ms = slice(mc * DH, (mc + 1) * DH)
                        psv = ppsum.tile([128, FT], f32, tag="pv", bufs=4)
                        for c in range(KC):
                            nc.tensor.matmul(
                                psv[:], mem_sb[:, c, ms], wv_sb[:, c, ts_],
                                start=(c == 0), stop=(c == KC - 1))
                        # bv folded into bf' on host -> pure copy drain (ACT
                        # is idle in phase 1; GpSimd cannot read PSUM)
                        nc.scalar.activation(v_sb[:, mc, ts_], psv[:], Identity)
                for h in range(H):
                    hs = slice(h * DH, (h + 1) * DH)
                    ps = ppsum.tile([128, Q], f32, tag="pp")
                    for c in range(KC):
                        for ts_ in m_tiles:
                            nc.tensor.matmul(
                                ps[:, ts_], wk_sb[:, c, hs], mem_sb[:, c, ts_],
                                start=(c == 0), stop=(c == KC - 1))
                    nc.scalar.activation(
                        k_sb[:, h, :], ps[:, :MKW], Identity, bias=bkt[:, h:h + 1])
                for h in range(H):
                    hs = slice(h * DH, (h + 1) * DH)
                    ps2 = ppsum.tile([128, Q], f32, tag="pp")
                    for c in range(KC):
                        for t in range(NT):
                            ts_ = slice(t * FT, (t + 1) * FT)
                            nc.tensor.matmul(
                                ps2[:, ts_], wq_sb[:, c, hs], qt_sb[:, c, ts_],
                                start=(c == 0), stop=(c == KC - 1))
                    nc.scalar.activation(
                        q_sb[:, h, :], ps2[:], Identity, bias=bqt[:, h:h + 1])

            # ---------------- Phase 2: attention (per head) ----------------
            if 'a' not in parts:
                continue
            with tc.tile_pool(name=f"attn{rep}", bufs=2) as attn, \
                 tc.tile_pool(name=f"attn3{rep}", bufs=3) as attn3, \
                 tc.tile_pool(name=f"dramp{rep}", bufs=2, space="DRAM") as dramp, \
                 tc.tile_pool(name=f"spsum{rep}", bufs=2, space="PSUM") as spsum, \
                 tc.tile_pool(name=f"cpsum{rep}", bufs=3, space="PSUM") as cpsum, \
                 tc.tile_pool(name=f"upsum{rep}", bufs=1, space="PSUM") as upsum:
                def emit_pv(ph, pexp, prb, cp_tiles, idx):
                    # one PV matmul of the software-pipelined previous head;
                    # idx walks cc-major: cc = idx // NT, t = idx % NT so the
                    # stationary V chunk is reused by consecutive matmuls
                    cc, t_ = divmod(idx, NT)
                    ts_ = slice(t_ * FT, (t_ + 1) * FT)
                    phs = slice(ph * DH, (ph + 1) * DH)
                    if idx == 0:
                        for t in range(NT):
                            cp_tiles[t] = cpsum.tile([128, FT], f32, tag="cp",
                                                     name=f"cp_h{ph}_t{t}")
                    nc.tensor.matmul(
                        cp_tiles[t_][:], v_sb[:, cc, phs], pexp[:, cc, ts_],
                        start=(cc == 0), stop=(cc == MKC - 1))
                    if cc == MKC - 1:
                        if norm:
                            nc.vector.tensor_mul(
                                ctx_sb[:, ph, ts_], cp_tiles[t_][:], prb[:, ts_])
                        else:
                            nc.vector.tensor_copy(ctx_sb[:, ph, ts_], cp_tiles[t_][:])

                def emit_p0(pexp, prb):
                    for c in range(MKC):
                        p0_sb = attn3.tile([128, Q], bf16, tag="p0")
                        nc.gpsimd.tensor_mul(p0_sb[:], pexp[:, c, :], prb[:])
                        nc.sync.dma_start(
                            out=p0t[c * DH:(c + 1) * DH, :], in_=p0_sb[:])

                def emit_sums(acc, rb_sb, h):
                    # deferred to the next head's window: the single upsum
                    # bank and the PE ones-matmuls then never block this
                    # head's own score stream in the in-order queues.
                    # 1/sum = exp(-ln(x)) on ACT (2 LUT passes, ~6x faster
                    # than DVE RECIPROCAL, which would also sit in front of
                    # the ctx drains in the in-order DVE queue).
                    for t in range(NT):
                        ts_ = slice(t * FT, (t + 1) * FT)
                        sum_bc = upsum.tile([128, FT], f32, tag="sum",
                                            name=f"sum_h{h}_t{t}")
                        nc.tensor.matmul(
                            sum_bc[:], ones128[:], acc[:, ts_],
                            start=True, stop=True)
                        ln_t = attn.tile([128, FT], f32, tag="ln")
                        nc.scalar.activation(ln_t[:], sum_bc[:], Ln)
                        nc.scalar.activation(
                            rb_sb[:, ts_], ln_t[:], Exp, scale=-1.0)

                prev = None       # (h, exp_sb, rb_sb)
                pend_sums = None  # (acc, rb_sb, h) for the previous head
                NP = MKC // 2  # exp chunk pairs
                for h in range(H):
                    hs = slice(h * DH, (h + 1) * DH)
                    exp_sb = attn.tile([128, MKC, Q], bf16, tag="expS",
                                       bufs=3 if fast else 2,
                                       name=f"exp_h{h}")
                    if norm:
                        red1 = attn.tile([128, NP, Q], bf16, tag="red1")
                    cp_tiles = [None, None]
                    for c in range(MKC):
                        cs = slice(c * DH, (c + 1) * DH)
                        st = spsum.tile([128, Q], f32, tag="st")
                        for t in range(NT):
                            ts_ = slice(t * FT, (t + 1) * FT)
                            nc.tensor.matmul(
                                st[:, ts_], k_sb[:, h, cs], q_sb[:, h, ts_],
                                start=True, stop=True)
                        if fast:
                            nc.scalar.activation(exp_sb[:, c, :], st[:], Exp)
                        else:
                            nc.scalar.activation(
                                exp_sb[:, c, :], st[:], Exp, bias=mbt[:, c:c + 1])
                        if norm and c % 2 == 1:
                            # incremental tree level 1: sum this exp pair as
                            # soon as both chunks exist (shortens the rb
                            # critical path after the last exp)
                            nc.vector.tensor_add(
                                red1[:, c // 2, :], exp_sb[:, c - 1, :],
                                exp_sb[:, c, :])
                        if c == 0 and pend_sums is not None:
                            emit_sums(*pend_sums)
                            pend_sums = None
                        if prev is not None:
                            # interleave prev head's PV so PE stays busy (and
                            # HAM-warm) through the ACT-bound exp stretch
                            emit_pv(prev[0], prev[1], prev[2], cp_tiles, 2 * c)
                            emit_pv(prev[0], prev[1], prev[2], cp_tiles, 2 * c + 1)
                    if prev is not None and prev[0] == 0 and norm:
                        emit_p0(prev[1], prev[2])
                    if norm:
                        # finish the pair tree, then a ones[128,128]-stationary
                        # matmul: every output partition gets the
                        # cross-partition sum, i.e. broadcast comes free.
                        if NP == 3:
                            red2 = attn.tile([128, Q], bf16, tag="red2")
                            nc.vector.tensor_add(
                                red2[:], red1[:, 0, :], red1[:, 1, :])
                            acc = attn.tile([128, Q], bf16, tag="acc")
                            nc.vector.tensor_add(acc[:], red2[:], red1[:, 2, :])
                        else:
                            red2 = attn.tile([128, 2, Q], bf16, tag="red2")
                            nc.vector.tensor_add(
                                red2[:], red1[:, 0:4:2, :], red1[:, 1:4:2, :])
                            acc = attn.tile([128, Q], bf16, tag="acc")
                            nc.vector.tensor_add(acc[:], red2[:, 0, :], red2[:, 1, :])
                        rb_sb = attn.tile([128, Q], f32, tag="rb", name=f"rb_h{h}")
                        pend_sums = (acc, rb_sb, h)
                    else:
                        rb_sb = None
                    prev = (h, exp_sb, rb_sb)

                # drain the last head: its sums first, then its PV
                if pend_sums is not None:
                    emit_sums(*pend_sums)
                cp_tiles = [None, None]
                for idx in range(NT * MKC):
                    emit_pv(prev[0], prev[1], prev[2], cp_tiles, idx)

            # ---------------- Phase 3: final projection ----------------
            if 'f' not in parts:
                continue
            with tc.tile_pool(name=f"fin{rep}", bufs=3) as fin, \
                 tc.tile_pool(name=f"fpsum{rep}", bufs=3, space="PSUM") as fpsum:
                for qc in range(KC):
                    qs = slice(qc * DH, (qc + 1) * DH)
                    for t in range(NT):
                        ts_ = slice(t * FT, (t + 1) * FT)
                        fp = fpsum.tile([128, FT], f32, tag="fp")
                        for h in range(H):
                            nc.tensor.matmul(
                                fp[:], ctx_sb[:, h, qs], wf_sb[:, h, ts_],
                                start=(h == 0), stop=(h == H - 1))
                        of = fin.tile([128, FT], bf16, tag="of")
                        nc.vector.tensor_add(of[:], fp[:], bft[:, ts_])
                        eng = nc.scalar if (qc + t) % 2 == 0 else nc.sync
                        eng.dma_start(out=wm[qs, ts_], in_=of[:])

    split_sync_waits(nc)
    return nc


def _get_program(reps=1, parts='paf', norm=True, fast=True):
    key = f"nc{reps}_{parts}_{norm}_{fast}"
    if key not in _CACHE:
        _CACHE[key] = _build_program(reps, parts, norm, fast)
    return _CACHE[key]


def _is_canonical_mask(mask):
    mask = np.asarray(mask)
    want = np.broadcast_to(np.arange(M)[None, :] >= MK_FAST, (B, M))
    return mask.shape == (B, M) and bool(np.array_equal(mask, want))


def _host_prep(query, memory, mask, Wk, bk, Wv, bv, Wq, bq, Wf, bf, fast):
    scale = 1.0 / math.sqrt(DH)
    f32 = np.float32
    mkw = MK_FAST if fast else M

    def pcn(t):
        # [D, N] -> partition-major [128, KC, N]: per-SBUF-partition data is
        # one contiguous DRAM run -> large DMA descriptors
        n = t.shape[1]
        return np.ascontiguousarray(
            t.reshape(KC, 128, n).transpose(1, 0, 2))

    def t_bf16(a):
        return pcn(np.asarray(a, dtype=f32).T.astype(_BF16))

    # bv folds through PV (softmax rows sum to 1): bf' = bf + Wf @ bv
    bf_eff = np.asarray(bf, dtype=f32) + (
        np.asarray(Wf, dtype=f32) @ np.asarray(bv, dtype=f32))

    shared = {
        "wkT": t_bf16(Wk),
        "wvT": t_bf16(Wv),
        "wqT": pcn((np.asarray(Wq, dtype=f32).T * f32(scale)).astype(_BF16)),
        "wfT": t_bf16(Wf),
        "bk_pp": np.ascontiguousarray(
            np.asarray(bk, dtype=f32).reshape(H, DH).T),
        "bq_pp": np.ascontiguousarray(
            (np.asarray(bq, dtype=f32) * f32(scale)).reshape(H, DH).T),
        "bf_bc": np.ascontiguousarray(
            np.broadcast_to(bf_eff, (128, D))),
    }
    mask = np.asarray(mask)
    in_maps = []
    for b in range(B):
        im = {
            **shared,
            "memT": t_bf16(np.asarray(memory[b])[:mkw]),
            "qT": t_bf16(query[b]),
        }
        if not fast:
            mb = np.where(mask[b], f32(-1e30), f32(0.0)).astype(f32)
            im["mb_pp"] = np.ascontiguousarray(mb.reshape(KC, DH).T)
        in_maps.append(im)
    return in_maps


def kernel(query, memory, mask, Wk, bk, Wv, bv, Wq, bq, Wf, bf):
    from concourse.bass_utils import run_bass_kernel_spmd

    fast = _is_canonical_mask(mask)
    nc = _get_program(fast=fast)
    in_maps = _host_prep(query, memory, mask, Wk, bk, Wv, bv, Wq, bq, Wf, bf,
                         fast)
    res = run_bass_kernel_spmd(nc, in_maps, core_ids=list(range(B)))
    mkw = MK_FAST if fast else M
    wm = np.stack([res.results[b]["wm"] for b in range(B)]).astype(np.float32)
    w0 = np.zeros((B, Q, M), dtype=np.float32)
    for b in range(B):
        w0[b, :, :mkw] = res.results[b]["p0t"].T.astype(np.float32)
    return wm, w0
